# revision 1
# baseline (speedup 1.0000x reference)
"""BoxHead MLP (Linear 12544->1024 + ReLU + Linear 1024->1024 + ReLU +
class/box heads) on 8 Trainium2 NeuronCores.

Strategy: data-parallel over the 16000 proposals (2000 per core), weights
replicated. All matmuls run with the activations kept transposed
(h^T: hidden on partitions, proposals on the free axis) so layer k's
output is directly consumable as layer k+1's moving operand -- no
on-device transposes. The feature matrix is transposed/cast to fp16 on
the host (host prep is not part of device execution time).

Per m-chunk of 500 proposals:
  layer1: 98 k-tiles x 8 hid-tiles matmuls (fp16, fp32 PSUM accum)
  evict:  ScalarE Relu(psum + b1) -> fp16 SBUF
  layer2: 8 x 8 matmuls (W2 resident in SBUF)
  heads:  Wc|Wr concatenated -> (16 x 500) PSUM, + bias -> fp32 out
"""

import numpy as np
from contextlib import ExitStack

N_CORES = 8
P = 128
D_IN = 12544
KT = D_IN // P            # 98 feature tiles
D_HID = 1024
HT = D_HID // P           # 8 hidden tiles
N_PROP = 16000
M_CORE = N_PROP // N_CORES  # 2000 proposals per core
M_CHUNK = 500
N_CHUNKS = M_CORE // M_CHUNK  # 4
NH = 16                   # 4 class logits + 12 box deltas

_CACHE = {}


def _build_nc():
    import concourse.mybir as mybir
    import concourse.tile as tile
    from concourse import bacc

    f16 = mybir.dt.float16
    f32 = mybir.dt.float32

    nc = bacc.Bacc("TRN2", target_bir_lowering=False, debug=False)

    xt_d = nc.dram_tensor("xt", [N_CHUNKS, D_IN, M_CHUNK], f16, kind="ExternalInput")
    w1_d = nc.dram_tensor("w1", [D_IN, D_HID], f16, kind="ExternalInput")
    b1_d = nc.dram_tensor("b1", [P, HT], f32, kind="ExternalInput")
    w2_d = nc.dram_tensor("w2", [HT, P, D_HID], f16, kind="ExternalInput")
    b2_d = nc.dram_tensor("b2", [P, HT], f32, kind="ExternalInput")
    wh_d = nc.dram_tensor("wh", [HT, P, NH], f16, kind="ExternalInput")
    bh_d = nc.dram_tensor("bh", [NH, 1], f32, kind="ExternalInput")
    out_d = nc.dram_tensor("out", [NH, M_CORE], f32, kind="ExternalOutput")

    Relu = mybir.ActivationFunctionType.Relu
    Ident = mybir.ActivationFunctionType.Identity

    with tile.TileContext(nc) as tc, ExitStack() as ctx:
        consts = ctx.enter_context(tc.tile_pool(name="consts", bufs=1))
        xt_pool = ctx.enter_context(tc.tile_pool(name="xt", bufs=24))
        w1_pool = ctx.enter_context(tc.tile_pool(name="w1", bufs=4))
        h_pool = ctx.enter_context(tc.tile_pool(name="h", bufs=2))
        h2_pool = ctx.enter_context(tc.tile_pool(name="h2", bufs=2))
        out_pool = ctx.enter_context(tc.tile_pool(name="out", bufs=2))
        psum_pool = ctx.enter_context(
            tc.tile_pool(name="psum", bufs=8, space="PSUM")
        )

        b1_t = consts.tile([P, HT], f32)
        nc.sync.dma_start(b1_t[:], b1_d[:, :])
        b2_t = consts.tile([P, HT], f32)
        nc.sync.dma_start(b2_t[:], b2_d[:, :])
        bh_t = consts.tile([NH, 1], f32)
        nc.sync.dma_start(bh_t[:], bh_d[:, :])
        wh_t = consts.tile([P, HT, NH], f16)
        nc.sync.dma_start(wh_t[:], wh_d.rearrange("t p n -> p t n"))
        w2_t = consts.tile([P, HT, D_HID], f16)
        nc.sync.dma_start(w2_t[:], w2_d.rearrange("t p n -> p t n"))

        for mi in range(N_CHUNKS):
            # ---- layer 1: psum1[h] += W1[k,h-tile]^T-load @ X^T[k, chunk]
            psum1 = [
                psum_pool.tile([P, 512], f32, name=f"ps1_{mi}_{h}", tag="ps")[
                    :, :M_CHUNK
                ]
                for h in range(HT)
            ]
            for k in range(KT):
                xt_t = xt_pool.tile([P, M_CHUNK], f16, tag="xt")
                nc.sync.dma_start(xt_t[:], xt_d[mi, k * P:(k + 1) * P, :])
                w1_t = w1_pool.tile([P, D_HID], f16, tag="w1")
                nc.sync.dma_start(w1_t[:], w1_d[k * P:(k + 1) * P, :])
                for h in range(HT):
                    nc.tensor.matmul(
                        psum1[h][:],
                        w1_t[:, h * P:(h + 1) * P],
                        xt_t[:],
                        start=(k == 0),
                        stop=(k == KT - 1),
                    )
            h_t = h_pool.tile([P, HT, M_CHUNK], f16, tag="h")
            for h in range(HT):
                nc.scalar.activation(
                    h_t[:, h, :], psum1[h][:], Relu, bias=b1_t[:, h:h + 1]
                )

            # ---- layer 2: psum2[h2] += W2[k2, h2-tile] @ h^T[k2, chunk]
            psum2 = [
                psum_pool.tile([P, 512], f32, name=f"ps2_{mi}_{h}", tag="ps")[
                    :, :M_CHUNK
                ]
                for h in range(HT)
            ]
            for k2 in range(HT):
                for h2 in range(HT):
                    nc.tensor.matmul(
                        psum2[h2][:],
                        w2_t[:, k2, h2 * P:(h2 + 1) * P],
                        h_t[:, k2, :],
                        start=(k2 == 0),
                        stop=(k2 == HT - 1),
                    )
            h2_t = h2_pool.tile([P, HT, M_CHUNK], f16, tag="h2")
            for h2 in range(HT):
                nc.scalar.activation(
                    h2_t[:, h2, :], psum2[h2][:], Relu, bias=b2_t[:, h2:h2 + 1]
                )

            # ---- heads: (16 x M_CHUNK)
            psum3 = psum_pool.tile([P, 512], f32, name=f"ps3_{mi}", tag="ps")[
                :NH, :M_CHUNK
            ]
            for k2 in range(HT):
                nc.tensor.matmul(
                    psum3[:],
                    wh_t[:, k2, :],
                    h2_t[:, k2, :],
                    start=(k2 == 0),
                    stop=(k2 == HT - 1),
                )
            out_t = out_pool.tile([NH, M_CHUNK], f32, tag="out")
            nc.scalar.activation(out_t[:], psum3[:], Ident, bias=bh_t[:, 0:1])
            nc.sync.dma_start(
                out_d[:, mi * M_CHUNK:(mi + 1) * M_CHUNK], out_t[:]
            )

    nc.compile()
    return nc


def _get_nc():
    if "nc" not in _CACHE:
        _CACHE["nc"] = _build_nc()
    return _CACHE["nc"]


def _prep_in_maps(feature_vectors, W1, b1, W2, b2, Wc, bc, Wr, br):
    f16 = np.float16
    f32 = np.float32

    w1_h = np.ascontiguousarray(W1, dtype=f16)
    b1_h = np.ascontiguousarray(np.asarray(b1, dtype=f32).reshape(HT, P).T)
    w2_h = np.ascontiguousarray(
        np.asarray(W2, dtype=f16).reshape(HT, P, D_HID)
    )
    b2_h = np.ascontiguousarray(np.asarray(b2, dtype=f32).reshape(HT, P).T)
    wcat = np.concatenate(
        [np.asarray(Wc, dtype=f32), np.asarray(Wr, dtype=f32)], axis=1
    )
    wh_h = np.ascontiguousarray(wcat.astype(f16).reshape(HT, P, NH))
    bcat = np.concatenate(
        [np.asarray(bc, dtype=f32), np.asarray(br, dtype=f32)]
    )
    bh_h = np.ascontiguousarray(bcat.reshape(NH, 1))

    x16 = np.asarray(feature_vectors, dtype=f16)
    in_maps = []
    for c in range(N_CORES):
        xc = x16[c * M_CORE:(c + 1) * M_CORE]  # (2000, 12544)
        xt = np.ascontiguousarray(
            xc.reshape(N_CHUNKS, M_CHUNK, D_IN).transpose(0, 2, 1)
        )  # (4, 12544, 500)
        in_maps.append({
            "xt": xt,
            "w1": w1_h,
            "b1": b1_h,
            "w2": w2_h,
            "b2": b2_h,
            "wh": wh_h,
            "bh": bh_h,
        })
    return in_maps


def _run(in_maps, trace=False):
    from concourse.bass_utils import run_bass_kernel_spmd

    nc = _get_nc()
    return run_bass_kernel_spmd(
        nc, in_maps, core_ids=list(range(N_CORES)), trace=trace
    )


def kernel(feature_vectors, W1, b1, W2, b2, Wc, bc, Wr, br):
    in_maps = _prep_in_maps(
        feature_vectors, W1, b1, W2, b2, Wc, bc, Wr, br
    )
    res = _run(in_maps, trace=False)
    out = np.concatenate(
        [res.results[c]["out"] for c in range(N_CORES)], axis=1
    )  # (16, 16000)
    class_logits = np.ascontiguousarray(out[0:4].T, dtype=np.float32)
    box_pred = np.ascontiguousarray(out[4:16].T, dtype=np.float32)
    return class_logits, box_pred


# revision 2
# speedup vs baseline: 1.0220x; 1.0220x over previous
"""BoxHead MLP (Linear 12544->1024 + ReLU + Linear 1024->1024 + ReLU +
class/box heads) on 8 Trainium2 NeuronCores.

Strategy: data-parallel over the 16000 proposals (2000 per core), weights
replicated. All matmuls run with the activations kept transposed
(h^T: hidden on partitions, proposals on the free axis) so layer k's
output is directly consumable as layer k+1's moving operand -- no
on-device transposes. The feature matrix is transposed/cast to fp16 on
the host (host prep is not part of device execution time).

Per m-chunk of 500 proposals:
  layer1: 98 k-tiles x 8 hid-tiles matmuls (fp16, fp32 PSUM accum)
  evict:  ScalarE Relu(psum + b1) -> fp16 SBUF
  layer2: 8 x 8 matmuls (W2 resident in SBUF)
  heads:  Wc|Wr concatenated -> (16 x 500) PSUM, + bias -> fp32 out

DMA layout: x^T and W1 are streamed two 128-row k-tiles per transfer
(250KB / 512KB) on the Sync HWDGE ring; resident tensors (W2, heads,
biases) load via the Scalar HWDGE ring so they don't delay the first
stream tiles.
"""

import numpy as np
from contextlib import ExitStack

N_CORES = 8
P = 128
D_IN = 12544
KT = D_IN // P            # 98 feature tiles
KT2 = KT // 2             # 49 double-tiles
D_HID = 1024
HT = D_HID // P           # 8 hidden tiles
N_PROP = 16000
M_CORE = N_PROP // N_CORES  # 2000 proposals per core
M_CHUNK = 500
N_CHUNKS = M_CORE // M_CHUNK  # 4
NH = 16                   # 4 class logits + 12 box deltas

_CACHE = {}


def _build_nc():
    import concourse.mybir as mybir
    import concourse.tile as tile
    from concourse import bacc

    f16 = mybir.dt.float16
    f32 = mybir.dt.float32

    nc = bacc.Bacc("TRN2", target_bir_lowering=False, debug=False)

    xt_d = nc.dram_tensor(
        "xt", [N_CHUNKS, KT2, 2, P, M_CHUNK], f16, kind="ExternalInput"
    )
    w1_d = nc.dram_tensor("w1", [KT2, 2, P, D_HID], f16, kind="ExternalInput")
    b1_d = nc.dram_tensor("b1", [P, HT], f32, kind="ExternalInput")
    w2_d = nc.dram_tensor("w2", [HT, P, D_HID], f16, kind="ExternalInput")
    b2_d = nc.dram_tensor("b2", [P, HT], f32, kind="ExternalInput")
    wh_d = nc.dram_tensor("wh", [HT, P, NH], f16, kind="ExternalInput")
    bh_d = nc.dram_tensor("bh", [NH, 1], f32, kind="ExternalInput")
    out_d = nc.dram_tensor("out", [NH, M_CORE], f32, kind="ExternalOutput")

    Relu = mybir.ActivationFunctionType.Relu
    Ident = mybir.ActivationFunctionType.Identity

    with tile.TileContext(nc) as tc, ExitStack() as ctx:
        consts = ctx.enter_context(tc.tile_pool(name="consts", bufs=1))
        xt_pool = ctx.enter_context(tc.tile_pool(name="xt", bufs=14))
        w1_pool = ctx.enter_context(tc.tile_pool(name="w1", bufs=5))
        h_pool = ctx.enter_context(tc.tile_pool(name="h", bufs=2))
        h2_pool = ctx.enter_context(tc.tile_pool(name="h2", bufs=2))
        out_pool = ctx.enter_context(tc.tile_pool(name="out", bufs=2))
        psum_pool = ctx.enter_context(
            tc.tile_pool(name="psum", bufs=8, space="PSUM")
        )

        # Residents + biases on the Scalar HWDGE ring (keeps the Sync ring
        # free for the first stream tiles).
        b1_t = consts.tile([P, HT], f32)
        nc.scalar.dma_start(b1_t[:], b1_d[:, :])
        b2_t = consts.tile([P, HT], f32)
        nc.scalar.dma_start(b2_t[:], b2_d[:, :])
        bh_t = consts.tile([NH, 1], f32)
        nc.scalar.dma_start(bh_t[:], bh_d[:, :])
        wh_t = consts.tile([P, HT, NH], f16)
        nc.scalar.dma_start(wh_t[:], wh_d.rearrange("t p n -> p t n"))
        w2_t = consts.tile([P, HT, D_HID], f16)
        nc.scalar.dma_start(w2_t[:], w2_d.rearrange("t p n -> p t n"))

        for mi in range(N_CHUNKS):
            # ---- layer 1: psum1[h] += W1[k,h-tile] stationary @ X^T[k,chunk]
            psum1 = [
                psum_pool.tile([P, 512], f32, name=f"ps1_{mi}_{h}", tag="ps")[
                    :, :M_CHUNK
                ]
                for h in range(HT)
            ]
            for kk in range(KT2):
                xt_t = xt_pool.tile([P, 2, M_CHUNK], f16, tag="xt")
                nc.sync.dma_start(
                    xt_t[:], xt_d[mi, kk].rearrange("t p m -> p t m")
                )
                w1_t = w1_pool.tile([P, 2, D_HID], f16, tag="w1")
                nc.sync.dma_start(
                    w1_t[:], w1_d[kk].rearrange("t p n -> p t n")
                )
                for t in range(2):
                    k = 2 * kk + t
                    for h in range(HT):
                        nc.tensor.matmul(
                            psum1[h][:],
                            w1_t[:, t, h * P:(h + 1) * P],
                            xt_t[:, t, :],
                            start=(k == 0),
                            stop=(k == KT - 1),
                        )
            h_t = h_pool.tile([P, HT, M_CHUNK], f16, tag="h")
            for h in range(HT):
                nc.scalar.activation(
                    h_t[:, h, :], psum1[h][:], Relu, bias=b1_t[:, h:h + 1]
                )

            # ---- layer 2: psum2[h2] += W2[k2, h2-tile] @ h^T[k2, chunk]
            psum2 = [
                psum_pool.tile([P, 512], f32, name=f"ps2_{mi}_{h}", tag="ps")[
                    :, :M_CHUNK
                ]
                for h in range(HT)
            ]
            for k2 in range(HT):
                for h2 in range(HT):
                    nc.tensor.matmul(
                        psum2[h2][:],
                        w2_t[:, k2, h2 * P:(h2 + 1) * P],
                        h_t[:, k2, :],
                        start=(k2 == 0),
                        stop=(k2 == HT - 1),
                    )
            h2_t = h2_pool.tile([P, HT, M_CHUNK], f16, tag="h2")
            for h2 in range(HT):
                nc.scalar.activation(
                    h2_t[:, h2, :], psum2[h2][:], Relu, bias=b2_t[:, h2:h2 + 1]
                )

            # ---- heads: (16 x M_CHUNK)
            psum3 = psum_pool.tile([P, 512], f32, name=f"ps3_{mi}", tag="ps")[
                :NH, :M_CHUNK
            ]
            for k2 in range(HT):
                nc.tensor.matmul(
                    psum3[:],
                    wh_t[:, k2, :],
                    h2_t[:, k2, :],
                    start=(k2 == 0),
                    stop=(k2 == HT - 1),
                )
            out_t = out_pool.tile([NH, M_CHUNK], f32, tag="out")
            nc.scalar.activation(out_t[:], psum3[:], Ident, bias=bh_t[:, 0:1])
            nc.sync.dma_start(
                out_d[:, mi * M_CHUNK:(mi + 1) * M_CHUNK], out_t[:]
            )

    nc.compile()
    return nc


def _get_nc():
    if "nc" not in _CACHE:
        _CACHE["nc"] = _build_nc()
    return _CACHE["nc"]


def _prep_in_maps(feature_vectors, W1, b1, W2, b2, Wc, bc, Wr, br):
    f16 = np.float16
    f32 = np.float32

    w1_h = np.ascontiguousarray(W1, dtype=f16).reshape(KT2, 2, P, D_HID)
    b1_h = np.ascontiguousarray(np.asarray(b1, dtype=f32).reshape(HT, P).T)
    w2_h = np.ascontiguousarray(
        np.asarray(W2, dtype=f16).reshape(HT, P, D_HID)
    )
    b2_h = np.ascontiguousarray(np.asarray(b2, dtype=f32).reshape(HT, P).T)
    wcat = np.concatenate(
        [np.asarray(Wc, dtype=f32), np.asarray(Wr, dtype=f32)], axis=1
    )
    wh_h = np.ascontiguousarray(wcat.astype(f16).reshape(HT, P, NH))
    bcat = np.concatenate(
        [np.asarray(bc, dtype=f32), np.asarray(br, dtype=f32)]
    )
    bh_h = np.ascontiguousarray(bcat.reshape(NH, 1))

    x16 = np.asarray(feature_vectors, dtype=f16)
    in_maps = []
    for c in range(N_CORES):
        xc = x16[c * M_CORE:(c + 1) * M_CORE]  # (2000, 12544)
        xt = np.ascontiguousarray(
            xc.reshape(N_CHUNKS, M_CHUNK, D_IN).transpose(0, 2, 1)
        ).reshape(N_CHUNKS, KT2, 2, P, M_CHUNK)
        in_maps.append({
            "xt": xt,
            "w1": w1_h,
            "b1": b1_h,
            "w2": w2_h,
            "b2": b2_h,
            "wh": wh_h,
            "bh": bh_h,
        })
    return in_maps


def _run(in_maps, trace=False):
    from concourse.bass_utils import run_bass_kernel_spmd

    nc = _get_nc()
    return run_bass_kernel_spmd(
        nc, in_maps, core_ids=list(range(N_CORES)), trace=trace
    )


def kernel(feature_vectors, W1, b1, W2, b2, Wc, bc, Wr, br):
    in_maps = _prep_in_maps(
        feature_vectors, W1, b1, W2, b2, Wc, bc, Wr, br
    )
    res = _run(in_maps, trace=False)
    out = np.concatenate(
        [res.results[c]["out"] for c in range(N_CORES)], axis=1
    )  # (16, 16000)
    class_logits = np.ascontiguousarray(out[0:4].T, dtype=np.float32)
    box_pred = np.ascontiguousarray(out[4:16].T, dtype=np.float32)
    return class_logits, box_pred
